# revision 1
# baseline (speedup 1.0000x reference)
"""FFF (fast feedforward / MoE-routing binary tree) forward pass on 8 Trainium2 NeuronCores.

Strategy (data-parallel over the 16384-token batch, 2048 tokens/core):
  - Levels 0..7 (255 nodes) are computed DENSE: logits via PE fp32 matmul,
    tree walk via one-hot map maintenance on DVE, masked acts @ w_out.T via PE.
  - Levels 8..11 (3840 nodes) are computed SPARSE: each token only needs one
    node per level, so we gather w_in rows by the walked node index
    (indirect DMA), form the logit with a fused multiply-reduce on DVE, and
    accumulate coef * w_outT[idx] into the same PSUM banks via a diagonal
    fp32 matmul on PE.
  Host pre-transposes x tiles / shallow weights so no on-device transposes of
  inputs are needed (PE only transposes the 255-wide masked activations).
"""

import numpy as np

P = 128
D = 1024
KC = 8                 # 1024 / 128 contraction chunks
N_NODES = 4095
SH_NODES = 255         # nodes in levels 0..7
SHN = 256              # padded
DEPTH = 11
N_CORES = 8
TOK = 2048             # tokens per core
NT = TOK // P          # 16 token tiles per core


def build_nc():
    import os
    from concourse import bacc, bass, mybir, tile
    from concourse.masks import make_identity

    stage = os.environ.get("KERNEL_STAGE", "full")
    deep_on = stage not in ("shallow",)
    batch_gather = stage in ("batchgather",)  # (128,4)-idx gather is broken on HW
    deep_mm_on = stage not in ("nodeepmm",)
    debug_dump = os.environ.get("KERNEL_DEBUG", "0") == "1"

    dt = mybir.dt
    AFT = mybir.ActivationFunctionType
    ALU = mybir.AluOpType

    nc = bacc.Bacc("TRN2", target_bir_lowering=False, debug=False)

    x_d = nc.dram_tensor("x", [TOK, D], dt.float32, kind="ExternalInput")
    xT_d = nc.dram_tensor("xT", [NT, KC, P, P], dt.float32, kind="ExternalInput")
    # wcat[n] = [w_in[n, :], w_outT[n, :]] — one 8KB gather serves both the
    # deep logit dot and the deep output accumulation.
    wcat_d = nc.dram_tensor("wcat", [N_NODES, 2 * D], dt.float32, kind="ExternalInput")
    w_inT_sh_d = nc.dram_tensor("w_inT_sh", [KC, P, SHN], dt.float32, kind="ExternalInput")
    woT_sh_d = nc.dram_tensor("woT_sh", [2, P, D], dt.float32, kind="ExternalInput")
    out_d = nc.dram_tensor("out", [TOK, D], dt.float32, kind="ExternalOutput")
    dbg = {}

    if debug_dump:
        dbg["logits"] = nc.dram_tensor("dbg_logits", [NT, P, SHN], dt.float32, kind="ExternalOutput")
        dbg["map"] = nc.dram_tensor("dbg_map", [NT, P, SHN], dt.float32, kind="ExternalOutput")
        dbg["mskT"] = nc.dram_tensor("dbg_mskT", [NT, P, 2 * P], dt.float32, kind="ExternalOutput")
        dbg["idx"] = nc.dram_tensor("dbg_idx", [NT, P, 4], dt.int32, kind="ExternalOutput")
        dbg["coef"] = nc.dram_tensor("dbg_coef", [NT, P, 4], dt.float32, kind="ExternalOutput")

    with tile.TileContext(nc) as tc:
        with (
            tc.tile_pool(name="const", bufs=1) as cpool,
            tc.tile_pool(name="xT", bufs=2) as xT_pool,
            tc.tile_pool(name="xn", bufs=5) as xn_pool,
            tc.tile_pool(name="small", bufs=4) as small_pool,
            tc.tile_pool(name="tiny", bufs=8) as tiny_pool,
            tc.tile_pool(name="mskT", bufs=4) as mskT_pool,
            tc.tile_pool(name="win", bufs=10) as win_pool,
            tc.tile_pool(name="dscr", bufs=2) as dscr_pool,
            tc.tile_pool(name="osb", bufs=3) as osb_pool,
            tc.tile_pool(name="lps", bufs=2, space="PSUM") as lps_pool,
            tc.tile_pool(name="tps", bufs=2, space="PSUM") as tps_pool,
            tc.tile_pool(name="ops", bufs=4, space="PSUM") as ops_pool,
        ):
            ident = cpool.tile([P, P], dt.float32)
            make_identity(nc, ident[:])
            w_inT_sb = cpool.tile([P, KC * SHN], dt.float32)
            nc.sync.dma_start(
                out=w_inT_sb[:].rearrange("p (k n) -> p k n", k=KC),
                in_=w_inT_sh_d[:].rearrange("k p n -> p k n"),
            )
            woT_sb = cpool.tile([P, 2 * D], dt.float32)
            nc.sync.dma_start(
                out=woT_sb[:].rearrange("p (c o) -> p c o", c=2),
                in_=woT_sh_d[:].rearrange("c p o -> p c o"),
            )

            for t in range(NT):
                xT = xT_pool.tile([P, D], dt.float32)
                nc.sync.dma_start(
                    out=xT[:].rearrange("p (k j) -> p k j", k=KC),
                    in_=xT_d[t].rearrange("k p j -> p k j"),
                )
                xn = xn_pool.tile([P, D], dt.float32)
                nc.sync.dma_start(out=xn[:], in_=x_d[t * P:(t + 1) * P, :])

                # ---- dense shallow logits: (128 tokens, 256 nodes) ----
                lps = lps_pool.tile([P, SHN], dt.float32, space="PSUM")
                for k in range(KC):
                    nc.tensor.matmul(
                        out=lps[:],
                        lhsT=xT[:, k * P:(k + 1) * P],
                        rhs=w_inT_sb[:, k * SHN:(k + 1) * SHN],
                        start=(k == 0),
                        stop=(k == KC - 1),
                    )
                lsb = small_pool.tile([P, SHN], dt.float32, tag="lsb")
                nc.scalar.copy(out=lsb[:], in_=lps[:])
                acts = small_pool.tile([P, SHN], dt.float32, tag="acts")
                nc.scalar.activation(out=acts[:], in_=lps[:], func=AFT.Gelu)

                # ---- shallow walk: one-hot decision map + heap index r ----
                mp = small_pool.tile([P, SHN], dt.float32, tag="map")
                nc.vector.memset(mp[:], 0.0)
                nc.vector.memset(mp[:, 0:1], 1.0)
                r = tiny_pool.tile([P, 1], dt.float32, tag="r")
                s2 = tiny_pool.tile([P, 1], dt.float32, tag="s2")
                pick = tiny_pool.tile([P, 1], dt.float32, tag="pick")
                dec = tiny_pool.tile([P, P], dt.float32, tag="dec")
                scr = tiny_pool.tile([P, P], dt.float32, tag="scr")
                # level 0: map[1]=1-dec0, map[2]=dec0, r=2+dec0
                nc.vector.tensor_scalar(
                    out=mp[:, 2:3], in0=lsb[:, 0:1], scalar1=0.0, scalar2=None, op0=ALU.is_gt
                )
                nc.vector.tensor_scalar(
                    out=mp[:, 1:2], in0=lsb[:, 0:1], scalar1=0.0, scalar2=None, op0=ALU.is_le
                )
                nc.vector.tensor_scalar(
                    out=r[:], in0=mp[:, 2:3], scalar1=2.0, scalar2=None, op0=ALU.add
                )
                for d in range(1, 8):
                    o = 2 ** d - 1
                    w = 2 ** d
                    nc.vector.tensor_scalar(
                        out=dec[:, :w], in0=lsb[:, o:o + w],
                        scalar1=0.0, scalar2=None, op0=ALU.is_gt,
                    )
                    nc.vector.tensor_scalar(
                        out=s2[:], in0=r[:], scalar1=2.0, scalar2=None, op0=ALU.mult
                    )
                    if d < 7:
                        o1 = 2 ** (d + 1) - 1
                        nxt = mp[:, o1:o1 + 2 * w].rearrange("p (n two) -> p n two", two=2)
                        # odd slots = OH*dec
                        nc.vector.tensor_tensor(
                            out=nxt[:, :, 1], in0=mp[:, o:o + w], in1=dec[:, :w],
                            op=ALU.mult,
                        )
                        nc.vector.tensor_reduce(
                            out=pick[:], in_=nxt[:, :, 1],
                            axis=mybir.AxisListType.X, op=ALU.add,
                        )
                        # even slots = OH - odd
                        nc.vector.tensor_tensor(
                            out=nxt[:, :, 0], in0=mp[:, o:o + w], in1=nxt[:, :, 1],
                            op=ALU.subtract,
                        )
                    else:
                        nc.vector.tensor_tensor(
                            out=scr[:, :w], in0=mp[:, o:o + w], in1=dec[:, :w],
                            op=ALU.mult,
                        )
                        nc.vector.tensor_reduce(
                            out=pick[:], in_=scr[:, :w],
                            axis=mybir.AxisListType.X, op=ALU.add,
                        )
                    nc.vector.tensor_tensor(out=r[:], in0=s2[:], in1=pick[:], op=ALU.add)

                # ---- masked acts + transpose for mm2 ----
                msk = small_pool.tile([P, SHN], dt.float32, tag="msk")
                nc.vector.tensor_tensor(out=msk[:], in0=acts[:], in1=mp[:], op=ALU.mult)
                mskT = mskT_pool.tile([P, 2 * P], dt.float32)
                for c in range(2):
                    tp = tps_pool.tile([P, P], dt.float32, space="PSUM")
                    nc.tensor.transpose(
                        out=tp[:], in_=msk[:, c * P:(c + 1) * P], identity=ident[:]
                    )
                    nc.scalar.copy(out=mskT[:, c * P:(c + 1) * P], in_=tp[:])

                # ---- deep levels 8..11: gather + fused dot + walk ----
                coef4 = tiny_pool.tile([P, 4], dt.float32, tag="coef4")
                idx4 = tiny_pool.tile([P, 4], dt.int32, tag="idx4")
                idxf = tiny_pool.tile([P, 1], dt.float32, tag="idxf")
                logit = tiny_pool.tile([P, 1], dt.float32, tag="logit")
                dscr = dscr_pool.tile([P, D], dt.float32)
                gws = []
                for l in range(4 if deep_on else 0):
                    dlev = 8 + l
                    nc.vector.tensor_scalar(
                        out=idxf[:], in0=r[:], scalar1=-1.0, scalar2=None, op0=ALU.add
                    )
                    nc.vector.tensor_copy(out=idx4[:, l:l + 1], in_=idxf[:])
                    gw = win_pool.tile([P, 2 * D], dt.float32)
                    gws.append(gw)
                    nc.gpsimd.indirect_dma_start(
                        out=gw[:],
                        out_offset=None,
                        in_=wcat_d[:],
                        in_offset=bass.IndirectOffsetOnAxis(ap=idx4[:, l:l + 1], axis=0),
                    )
                    nc.vector.tensor_tensor(
                        out=dscr[:], in0=xn[:], in1=gw[:, 0:D], op=ALU.mult
                    )
                    nc.vector.tensor_reduce(
                        out=logit[:], in_=dscr[:],
                        axis=mybir.AxisListType.X, op=ALU.add,
                    )
                    nc.scalar.activation(out=coef4[:, l:l + 1], in_=logit[:], func=AFT.Gelu)
                    if dlev < DEPTH:
                        nc.vector.tensor_scalar(
                            out=dec[:, 0:1], in0=logit[:], scalar1=0.0, scalar2=None,
                            op0=ALU.is_gt,
                        )
                        nc.vector.tensor_scalar(
                            out=s2[:], in0=r[:], scalar1=2.0, scalar2=None, op0=ALU.mult
                        )
                        nc.vector.tensor_tensor(
                            out=r[:], in0=s2[:], in1=dec[:, 0:1], op=ALU.add
                        )

                # ---- output accumulation in PSUM (one half-bank per N-half,
                # emitted after the deep chain so the bank is held briefly) ----
                dmm = deep_on and deep_mm_on
                diags = []
                for l in range(4 if dmm else 0):
                    dg = tiny_pool.tile([P, P], dt.float32, tag="diag")
                    nc.vector.tensor_scalar(
                        out=dg[:], in0=ident[:], scalar1=coef4[:, l:l + 1],
                        scalar2=None, op0=ALU.mult,
                    )
                    diags.append(dg)
                osb = osb_pool.tile([P, D], dt.float32)
                for h in range(2):
                    ops = ops_pool.tile([P, 512], dt.float32, space="PSUM")
                    for c in range(2):
                        nc.tensor.matmul(
                            out=ops[:],
                            lhsT=mskT[:, c * P:(c + 1) * P],
                            rhs=woT_sb[:, c * D + h * 512: c * D + h * 512 + 512],
                            start=(c == 0),
                            stop=(c == 1 and not dmm),
                            skip_group_check=True,
                        )
                    for l in range(4 if dmm else 0):
                        nc.tensor.matmul(
                            out=ops[:],
                            lhsT=diags[l][:],
                            rhs=gws[l][:, D + h * 512: D + h * 512 + 512],
                            start=False,
                            stop=(l == 3),
                            skip_group_check=True,
                        )
                    nc.scalar.copy(out=osb[:, h * 512:(h + 1) * 512], in_=ops[:])
                nc.sync.dma_start(out=out_d[t * P:(t + 1) * P, :], in_=osb[:])

                if debug_dump:
                    nc.sync.dma_start(out=dbg["logits"][t], in_=lsb[:])
                    nc.sync.dma_start(out=dbg["map"][t], in_=mp[:])
                    nc.sync.dma_start(out=dbg["mskT"][t], in_=mskT[:])
                    nc.sync.dma_start(out=dbg["idx"][t], in_=idx4[:])
                    nc.sync.dma_start(out=dbg["coef"][t], in_=coef4[:])

    nc.compile()
    return nc


def host_prep(x, w_in, w_out):
    """Build the per-core input maps (host-side transposes/tilings)."""
    x = np.ascontiguousarray(x, np.float32)
    w_in = np.ascontiguousarray(w_in, np.float32)
    w_out = np.ascontiguousarray(w_out, np.float32)

    w_inT_sh = np.zeros((SHN, D), np.float32)
    w_inT_sh[:SH_NODES] = w_in[:SH_NODES]
    w_inT_sh = np.ascontiguousarray(
        w_inT_sh.T.reshape(KC, P, SHN)
    )  # [k,p,n] = w_in[n, k*128+p]

    woT_sh = np.zeros((SHN, D), np.float32)
    woT_sh[:SH_NODES] = w_out[:, :SH_NODES].T
    woT_sh = np.ascontiguousarray(woT_sh.reshape(2, P, D))  # [c,p,o] = w_out[o, c*128+p]

    wcat = np.ascontiguousarray(
        np.concatenate([w_in, w_out.T], axis=1)
    )  # (4095, 2048): [w_in row | w_outT row]

    in_maps = []
    for c in range(N_CORES):
        xs = x[c * TOK:(c + 1) * TOK]
        xT = np.ascontiguousarray(
            xs.reshape(NT, P, KC, P).transpose(0, 2, 3, 1)
        )  # [t,k,p,j] = xs[t*128+j, k*128+p]
        in_maps.append(
            {
                "x": np.ascontiguousarray(xs),
                "xT": xT,
                "wcat": wcat,
                "w_inT_sh": w_inT_sh,
                "woT_sh": woT_sh,
            }
        )
    return in_maps


_NC_CACHE = {}


def kernel(x, w_in, w_out, force_depth=None, **_ignored):
    from concourse.bass_utils import run_bass_kernel_spmd

    if "nc" not in _NC_CACHE:
        _NC_CACHE["nc"] = build_nc()
    nc = _NC_CACHE["nc"]

    in_maps = host_prep(np.asarray(x), np.asarray(w_in), np.asarray(w_out))
    res = run_bass_kernel_spmd(nc, in_maps, core_ids=list(range(N_CORES)))
    out = np.concatenate([res.results[c]["out"] for c in range(N_CORES)], axis=0)
    return out.astype(np.float32)


if __name__ == "__main__":
    import reference

    inputs = reference.setup_inputs()
    expected = np.asarray(reference.reference(**inputs))
    actual = kernel(**{k: np.asarray(v) for k, v in inputs.items()})
    err = np.abs(actual - expected).max()
    print("absmax err:", err)



# revision 5
# speedup vs baseline: 1.3535x; 1.3535x over previous
"""FFF (fast feedforward / MoE-routing binary tree) forward pass on 8 Trainium2 NeuronCores.

Strategy (data-parallel over the 16384-token batch, 2048 tokens/core):
  - Levels 0..9 (1023 nodes) are computed DENSE: logits via PE fp32 matmul,
    tree walk via one-hot map maintenance on DVE (bf16), masked acts @ w_out.T
    via PE in bf16.
  - Levels 10..11 (3072 nodes) are computed SPARSE with ONE gather per token:
    after the walk reaches level 10, a single indirect DMA pulls a per-node
    blob holding [w_in(n10) fp32 | w_out(n10), w_in(children), w_out(children)
    bf16].  The level-10 logit is a fp32 DVE dot (decision-grade); both leaf
    children's logits are bf16 dots and the wrong child is zeroed via its
    coefficient, so no data-dependent select is needed.  Contributions enter
    the same output PSUM banks via diagonal bf16 matmuls.
  - Node numbering is relabeled (within-level bit-reversal, level-d block at
    free-dim offset 2^d) so every walk update is a pair of CONTIGUOUS
    tensor_tensor ops (bf16 2x mode) and the level-10 index is recovered with
    a single iota reduction instead of per-level pick extraction.
"""

import numpy as np

P = 128
D = 1024
KC = 8                 # 1024 / 128 contraction chunks
DEPTH = 11
DN = 1024              # dense slots: levels 0..9 (1023 nodes) + 1 pad at slot 0
N_CORES = 8
TOK = 2048             # tokens per core
NT = TOK // P          # 16 token tiles per core
BW = 5 * D             # blobB row width (bf16 words)


def build_nc():
    from concourse import bacc, bass, mybir, tile
    from concourse.masks import make_identity

    dt = mybir.dt
    AFT = mybir.ActivationFunctionType
    ALU = mybir.AluOpType

    nc = bacc.Bacc("TRN2", target_bir_lowering=False, debug=False)

    x_d = nc.dram_tensor("x", [TOK, D], dt.float32, kind="ExternalInput")
    xT_d = nc.dram_tensor("xT", [NT, KC, P, P], dt.float32, kind="ExternalInput")
    w_inT_d = nc.dram_tensor("w_inT_dn", [KC, P, DN], dt.float32, kind="ExternalInput")
    woT_d = nc.dram_tensor("woT_dn", [KC, P, D], dt.bfloat16, kind="ExternalInput")
    blobA_d = nc.dram_tensor("blobA", [DN, D], dt.float32, kind="ExternalInput")
    blobB_d = nc.dram_tensor("blobB", [DN, BW], dt.bfloat16, kind="ExternalInput")
    out_d = nc.dram_tensor("out", [TOK, D], dt.float32, kind="ExternalOutput")

    from contextlib import ExitStack

    with tile.TileContext(nc) as tc, ExitStack() as es:
        pool_specs = [
            ("const", 1, None), ("xT", 2, None), ("xn", 3, None),
            ("xnh", 3, None), ("nmap", 2, None), ("dec", 2, None),
            ("acts", 2, None), ("msk", 2, None), ("mskT", 2, None),
            ("gwA", 3, None), ("gwB", 3, None), ("dscr", 2, None),
            ("prods", 2, None), ("osb", 2, None), ("tiny", 4, None),
            ("lps", 2, "PSUM"), ("tps", 2, "PSUM"), ("ops", 1, "PSUM"),
        ]
        pools = {}
        for pname, bufs, spc in pool_specs:
            kw = {"name": pname, "bufs": bufs}
            if spc is not None:
                kw["space"] = spc
            pools[pname] = es.enter_context(tc.tile_pool(**kw))
        (cpool, xT_pool, xn_pool, xnh_pool, map_pool, dec_pool, acts_pool,
         msk_pool, mskT_pool, gwA_pool, gwB_pool, dscr_pool, prods_pool,
         osb_pool, tiny_pool, lps_pool, tps_pool, ops_pool) = (
            pools[n] for n, _, _ in pool_specs)
        if True:
            identb = cpool.tile([P, P], dt.bfloat16)
            make_identity(nc, identb[:])
            iotaf = cpool.tile([P, 512], dt.float32)
            nc.gpsimd.iota(
                iotaf[:], pattern=[[1, 512]], base=0, channel_multiplier=0,
                allow_small_or_imprecise_dtypes=True,
            )
            w_inT_sb = cpool.tile([P, KC * DN], dt.float32)
            nc.sync.dma_start(
                out=w_inT_sb[:].rearrange("p (k n) -> p k n", k=KC),
                in_=w_inT_d[:].rearrange("k p n -> p k n"),
            )
            woT_sb = cpool.tile([P, KC * D], dt.bfloat16)
            nc.sync.dma_start(
                out=woT_sb[:].rearrange("p (c o) -> p c o", c=KC),
                in_=woT_d[:].rearrange("c p o -> p c o"),
            )

            for t in range(NT):
                xT = xT_pool.tile([P, D], dt.float32)
                nc.sync.dma_start(
                    out=xT[:].rearrange("p (k j) -> p k j", k=KC),
                    in_=xT_d[t].rearrange("k p j -> p k j"),
                )
                xn = xn_pool.tile([P, D], dt.float32)
                nc.sync.dma_start(out=xn[:], in_=x_d[t * P:(t + 1) * P, :])
                xnh = xnh_pool.tile([P, D], dt.bfloat16)
                nc.scalar.copy(out=xnh[:], in_=xn[:])

                # ---- dense logits for levels 0..9: (128 tokens, 1024 slots) ----
                lps = lps_pool.tile([P, DN], dt.float32, space="PSUM")
                for k in range(KC):
                    for h in range(2):
                        nc.tensor.matmul(
                            out=lps[:, h * 512:(h + 1) * 512],
                            lhsT=xT[:, k * P:(k + 1) * P],
                            rhs=w_inT_sb[:, k * DN + h * 512: k * DN + (h + 1) * 512],
                            start=(k == 0),
                            stop=(k == KC - 1),
                        )
                dec = dec_pool.tile([P, DN], dt.bfloat16)
                nc.vector.tensor_scalar(
                    out=dec[:], in0=lps[:], scalar1=0.0, scalar2=None, op0=ALU.is_gt
                )
                acts = acts_pool.tile([P, DN], dt.bfloat16)
                nc.scalar.activation(out=acts[:], in_=lps[:], func=AFT.Gelu)

                # ---- walk: one-hot map, level-d block at [2^d, 2^{d+1}) ----
                mp = map_pool.tile([P, DN], dt.bfloat16)
                nc.vector.memset(mp[:, 0:1], 0.0)
                nc.vector.memset(mp[:, 1:2], 1.0)
                # level 0 -> 1: map[3]=dec[1], map[2]=1-dec[1]
                nc.vector.tensor_copy(out=mp[:, 3:4], in_=dec[:, 1:2])
                nc.vector.tensor_scalar(
                    out=mp[:, 2:3], in0=dec[:, 1:2],
                    scalar1=-1.0, scalar2=1.0, op0=ALU.mult, op1=ALU.add,
                )
                for d in range(1, 9):
                    w = 2 ** d
                    nc.vector.tensor_tensor(
                        out=mp[:, 3 * w:4 * w], in0=mp[:, w:2 * w],
                        in1=dec[:, w:2 * w], op=ALU.mult,
                    )
                    nc.vector.tensor_tensor(
                        out=mp[:, 2 * w:3 * w], in0=mp[:, w:2 * w],
                        in1=mp[:, 3 * w:4 * w], op=ALU.subtract,
                    )

                # ---- level-10 index: i10 = i9 + 512*dec9(on-path) ----
                m9f = tiny_pool.tile([P, 512], dt.float32, tag="m9f")
                nc.vector.tensor_copy(out=m9f[:], in_=mp[:, 512:1024])
                t1 = tiny_pool.tile([P, 512], dt.float32, tag="t1")
                nc.vector.tensor_tensor(
                    out=t1[:], in0=m9f[:], in1=iotaf[:], op=ALU.mult
                )
                i9 = tiny_pool.tile([P, 1], dt.float32, tag="i9")
                nc.vector.tensor_reduce(
                    out=i9[:], in_=t1[:], axis=mybir.AxisListType.X, op=ALU.add
                )
                t2 = tiny_pool.tile([P, 512], dt.bfloat16, tag="t2")
                nc.vector.tensor_tensor(
                    out=t2[:], in0=mp[:, 512:1024], in1=dec[:, 512:1024], op=ALU.mult
                )
                d9 = tiny_pool.tile([P, 1], dt.float32, tag="d9")
                nc.vector.tensor_reduce(
                    out=d9[:], in_=t2[:], axis=mybir.AxisListType.X, op=ALU.add
                )
                i10f = tiny_pool.tile([P, 1], dt.float32, tag="i10f")
                nc.vector.tensor_scalar(
                    out=i10f[:], in0=d9[:], scalar1=512.0, scalar2=None, op0=ALU.mult
                )
                nc.vector.tensor_tensor(
                    out=i10f[:], in0=i10f[:], in1=i9[:], op=ALU.add
                )
                idx = tiny_pool.tile([P, 1], dt.int32, tag="idx")
                nc.vector.tensor_copy(out=idx[:], in_=i10f[:])

                # ---- one gather for levels 10+11 ----
                gwA = gwA_pool.tile([P, D], dt.float32)
                nc.gpsimd.indirect_dma_start(
                    out=gwA[:], out_offset=None, in_=blobA_d[:],
                    in_offset=bass.IndirectOffsetOnAxis(ap=idx[:], axis=0),
                )
                gwB = gwB_pool.tile([P, BW], dt.bfloat16)
                nc.gpsimd.indirect_dma_start(
                    out=gwB[:], out_offset=None, in_=blobB_d[:],
                    in_offset=bass.IndirectOffsetOnAxis(ap=idx[:], axis=0),
                )

                # ---- level-10 fp32 dot -> decision + coef ----
                dscr = dscr_pool.tile([P, D], dt.float32)
                nc.vector.tensor_tensor(
                    out=dscr[:], in0=xn[:], in1=gwA[:], op=ALU.mult
                )
                l10 = tiny_pool.tile([P, 1], dt.float32, tag="l10")
                nc.vector.tensor_reduce(
                    out=l10[:], in_=dscr[:], axis=mybir.AxisListType.X, op=ALU.add
                )
                dec10 = tiny_pool.tile([P, 1], dt.float32, tag="dec10")
                nc.vector.tensor_scalar(
                    out=dec10[:], in0=l10[:], scalar1=0.0, scalar2=None, op0=ALU.is_gt
                )
                c10 = tiny_pool.tile([P, 1], dt.float32, tag="c10")
                nc.scalar.activation(out=c10[:], in_=l10[:], func=AFT.Gelu)

                # ---- both leaf children's bf16 dots; zero the unchosen one ----
                prods = prods_pool.tile([P, 2 * D], dt.bfloat16)
                nc.vector.tensor_tensor(
                    out=prods[:, 0:D], in0=xnh[:], in1=gwB[:, D:2 * D], op=ALU.mult
                )
                nc.vector.tensor_tensor(
                    out=prods[:, D:2 * D], in0=xnh[:], in1=gwB[:, 2 * D:3 * D],
                    op=ALU.mult,
                )
                clr = tiny_pool.tile([P, 2], dt.float32, tag="clr")
                nc.vector.tensor_reduce(
                    out=clr[:],
                    in_=prods[:].rearrange("p (c d) -> p c d", c=2),
                    axis=mybir.AxisListType.X, op=ALU.add,
                )
                cLR = tiny_pool.tile([P, 2], dt.float32, tag="cLR")
                nc.scalar.activation(out=cLR[:], in_=clr[:], func=AFT.Gelu)
                cl = tiny_pool.tile([P, 1], dt.float32, tag="cl")
                nc.vector.tensor_scalar(
                    out=cl[:], in0=dec10[:],
                    scalar1=-1.0, scalar2=1.0, op0=ALU.mult, op1=ALU.add,
                )
                nc.vector.tensor_tensor(
                    out=cl[:], in0=cl[:], in1=cLR[:, 0:1], op=ALU.mult
                )
                cr = tiny_pool.tile([P, 1], dt.float32, tag="cr")
                nc.vector.tensor_tensor(
                    out=cr[:], in0=dec10[:], in1=cLR[:, 1:2], op=ALU.mult
                )

                # ---- mask + transpose (bf16) ----
                msk = msk_pool.tile([P, DN], dt.bfloat16)
                nc.vector.tensor_tensor(
                    out=msk[:], in0=acts[:], in1=mp[:], op=ALU.mult
                )
                tps = tps_pool.tile([P, DN], dt.bfloat16, space="PSUM")
                for c in range(KC):
                    nc.tensor.transpose(
                        out=tps[:, c * P:(c + 1) * P],
                        in_=msk[:, c * P:(c + 1) * P],
                        identity=identb[:],
                    )
                mskT = mskT_pool.tile([P, DN], dt.bfloat16)
                nc.scalar.copy(out=mskT[:], in_=tps[:])

                # ---- diagonal coef tiles for the sparse contributions ----
                dg10 = tiny_pool.tile([P, P], dt.bfloat16, tag="dg10")
                nc.vector.tensor_scalar(
                    out=dg10[:], in0=identb[:], scalar1=c10[:], scalar2=None,
                    op0=ALU.mult,
                )
                dgl = tiny_pool.tile([P, P], dt.bfloat16, tag="dgl")
                nc.vector.tensor_scalar(
                    out=dgl[:], in0=identb[:], scalar1=cl[:], scalar2=None,
                    op0=ALU.mult,
                )
                dgr = tiny_pool.tile([P, P], dt.bfloat16, tag="dgr")
                nc.vector.tensor_scalar(
                    out=dgr[:], in0=identb[:], scalar1=cr[:], scalar2=None,
                    op0=ALU.mult,
                )

                # ---- output accumulation: dense 8 chunks + 3 sparse diags ----
                ops = ops_pool.tile([P, D], dt.float32, space="PSUM")
                for h in range(2):
                    o0 = h * 512
                    for c in range(KC):
                        nc.tensor.matmul(
                            out=ops[:, o0:o0 + 512],
                            lhsT=mskT[:, c * P:(c + 1) * P],
                            rhs=woT_sb[:, c * D + o0: c * D + o0 + 512],
                            start=(c == 0),
                            stop=False,
                            skip_group_check=True,
                        )
                    for dg, boff in ((dg10, 0), (dgl, 3 * D), (dgr, 4 * D)):
                        nc.tensor.matmul(
                            out=ops[:, o0:o0 + 512],
                            lhsT=dg[:],
                            rhs=gwB[:, boff + o0: boff + o0 + 512],
                            start=False,
                            stop=(boff == 4 * D),
                            skip_group_check=True,
                        )
                osb = osb_pool.tile([P, D], dt.float32)
                nc.scalar.copy(out=osb[:], in_=ops[:])
                nc.sync.dma_start(out=out_d[t * P:(t + 1) * P, :], in_=osb[:])

    nc.compile()
    return nc


def _bitrev(i, bits):
    r = 0
    for _ in range(bits):
        r = (r << 1) | (i & 1)
        i >>= 1
    return r


def _dense_perm():
    """perm[s] = heap node id stored at dense slot s (slot 0 unused)."""
    perm = np.zeros(DN, np.int64)
    for d in range(10):
        w = 2 ** d
        i = np.arange(w)
        rev = np.array([_bitrev(int(j), d) for j in i], np.int64)
        perm[w + i] = (w - 1) + rev
    return perm


def _leaf_perm():
    """lperm[i] = heap id of the level-10 node stored at blob row i."""
    i = np.arange(1024)
    rev = np.array([_bitrev(int(j), 10) for j in i], np.int64)
    return 1023 + rev


_DENSE_PERM = _dense_perm()
_LEAF_PERM = _leaf_perm()


def host_prep(x, w_in, w_out):
    """Build the per-core input maps (host-side transposes/tilings)."""
    import ml_dtypes

    bf16 = ml_dtypes.bfloat16
    x = np.ascontiguousarray(x, np.float32)
    w_in = np.ascontiguousarray(w_in, np.float32)
    w_out = np.ascontiguousarray(w_out, np.float32)

    # dense (levels 0..9) weights in shifted-relabeled order; slot 0 = zeros
    w_in_dn = np.zeros((DN, D), np.float32)
    w_in_dn[1:] = w_in[_DENSE_PERM[1:]]
    w_inT_dn = np.ascontiguousarray(
        w_in_dn.T.reshape(KC, P, DN)
    )  # [k,p,n] = w_in_dn[n, k*128+p]

    w_outT = np.ascontiguousarray(w_out.T)  # (n_nodes, D)
    woT_dn = np.zeros((DN, D), np.float32)
    woT_dn[1:] = w_outT[_DENSE_PERM[1:]]
    woT_dn = np.ascontiguousarray(
        woT_dn.reshape(KC, P, D).astype(bf16)
    )  # [c,p,o] = w_out_dn[o, c*128+p] in bf16

    n10 = _LEAF_PERM
    lc = 2 * n10 + 1
    rc = 2 * n10 + 2
    blobA = np.ascontiguousarray(w_in[n10])  # (1024, D) fp32
    blobB = np.ascontiguousarray(
        np.concatenate(
            [w_outT[n10], w_in[lc], w_in[rc], w_outT[lc], w_outT[rc]], axis=1
        ).astype(bf16)
    )  # (1024, 5D) bf16

    in_maps = []
    for c in range(N_CORES):
        xs = x[c * TOK:(c + 1) * TOK]
        xT = np.ascontiguousarray(
            xs.reshape(NT, P, KC, P).transpose(0, 2, 3, 1)
        )  # [t,k,p,j] = xs[t*128+j, k*128+p]
        in_maps.append(
            {
                "x": np.ascontiguousarray(xs),
                "xT": xT,
                "w_inT_dn": w_inT_dn,
                "woT_dn": woT_dn,
                "blobA": blobA,
                "blobB": blobB,
            }
        )
    return in_maps


_NC_CACHE = {}


def kernel(x, w_in, w_out, force_depth=None, **_ignored):
    from concourse.bass_utils import run_bass_kernel_spmd

    if "nc" not in _NC_CACHE:
        _NC_CACHE["nc"] = build_nc()
    nc = _NC_CACHE["nc"]

    in_maps = host_prep(np.asarray(x), np.asarray(w_in), np.asarray(w_out))
    res = run_bass_kernel_spmd(nc, in_maps, core_ids=list(range(N_CORES)))
    out = np.concatenate([res.results[c]["out"] for c in range(N_CORES)], axis=0)
    return out.astype(np.float32)


if __name__ == "__main__":
    import reference

    inputs = reference.setup_inputs()
    expected = np.asarray(reference.reference(**inputs))
    actual = kernel(**{k: np.asarray(v) for k, v in inputs.items()})
    err = np.abs(actual - expected).max()
    print("absmax err:", err)
